# revision 1
# baseline (speedup 1.0000x reference)
"""Multi-head attention (RoPE + causal softmax + out-proj) on 8 TRN2 NeuronCores.

Sharding: core c handles batch b = c // 2 and head-half g = c % 2 (8 of 16
heads). Each core computes q/k/v projections for its heads, RoPE, causal
attention, and a partial transposed output projection
outT = (y_heads @ Wo_part.T).T; the host sums the two partials per batch.

Device layout notes:
 - All matmul operands are float32r (single-pass full-rate PE fp32).
 - q/k weight rows are permuted host-side so the RoPE even/odd pairs become
   contiguous 32-partition blocks: tiles hold [x1 of 4 heads | x2 of 4 heads]
   and RoPE runs as full-width vector ops.  Attention contracts over dh=64 as
   two K=32 matmul passes (x1, x2) per head; two heads run concurrently on
   distinct 32-row PE groups via tile_position.
 - Attention is k-major: sT = k q^T tiles [k:128, q:512]; exp on ScalarE
   (1/sqrt(dh) folded into the activation scale); causal handling is
   tile-level skipping plus a precomputed 0/1 mask multiply (on GPSIMD) for
   diagonal tiles; y^T = v_aug.T @ expT where v_aug carries a ones column
   per head, making row 64 of y^T the softmax denominator for free.
 - Normalization is per-head-pair (reciprocal chunks + K=1 PE broadcast
   matmul + DVE multiply) so it overlaps the next pair's attention.
"""

import numpy as np

B, T, C, H = 4, 2048, 1024, 16
DH = C // H  # 64
NCORES = 8
HPC = H // 2  # 8 heads per core
QR = HPC * DH  # 512 rows per q/k/v section
TS = 512  # t-chunk width
NTS = T // TS  # 4
CC = C // 128  # 8 contraction chunks
NKT = T // 128  # 16 k-tiles / t-row chunks

_CACHE = {}


def _build_program():
    import concourse.mybir as mybir
    import concourse.tile as tile
    from concourse import bacc

    f32 = mybir.dt.float32
    f32r = mybir.dt.float32r
    EXP = mybir.ActivationFunctionType.Exp

    nc = bacc.Bacc(trn_type="TRN2")

    xT = nc.dram_tensor("xT", [C, T], f32, kind="ExternalInput").ap()
    wqkvT = nc.dram_tensor("wqkvT", [C, 3 * QR], f32, kind="ExternalInput").ap()
    woT = nc.dram_tensor("woT", [QR, C], f32, kind="ExternalInput").ap()
    cosT = nc.dram_tensor("cosT", [128, T], f32, kind="ExternalInput").ap()
    sinT = nc.dram_tensor("sinT", [128, T], f32, kind="ExternalInput").ap()
    maskd = nc.dram_tensor("maskd", [128, TS + 128], f32, kind="ExternalInput").ap()
    outT = nc.dram_tensor("outT", [C, T], f32, kind="ExternalOutput").ap()

    with tile.TileContext(nc) as tc:
        with tc.tile_pool(name="persist", bufs=1) as pp:
            # rope'd q/k in projection layout: key (part, grp, half):
            # rows 32*i = x-half of local head 4*grp+i
            qk = {}
            for part in range(2):
                for grp in range(2):
                    for half in range(2):
                        nm = f"qk{part}{grp}{half}"
                        qk[(part, grp, half)] = pp.tile(
                            [128, T], f32r, tag=nm, name=nm
                        )
            # v with a ones column per head: [t-chunk 128, 8 * 65]
            v_aug = [
                pp.tile([128, HPC * 65], f32r, tag=f"va{t}", name=f"va{t}")
                for t in range(NKT)
            ]

            # ---------------- phase A: qkv projection + rope ----------------
            with (
                tc.tile_pool(name="wpool", bufs=1) as wp,
                tc.tile_pool(name="xpool", bufs=9) as xp,
                tc.tile_pool(name="trig", bufs=1) as tp,
                tc.tile_pool(name="ropetmp", bufs=6) as rt,
                tc.tile_pool(name="psA", bufs=4, space="PSUM") as psA,
            ):
                # first weight chunk, then first x chunk set, then the rest —
                # so the first matmul group isn't stuck behind 6 MB of weights
                wtiles = [None] * CC
                w = wp.tile([128, 3 * QR], f32r, tag="w0", name="w0")
                nc.sync.dma_start(w[:], wqkvT[0:128, :].bitcast(f32r))
                wtiles[0] = w
                xts0 = []
                for cc in range(CC):
                    xt = xp.tile([128, TS], f32r, tag="xts", name=f"x0{cc}")
                    nc.sync.dma_start(
                        xt[:], xT[128 * cc : 128 * (cc + 1), 0:TS].bitcast(f32r)
                    )
                    xts0.append(xt)
                for cc in range(1, CC):
                    w = wp.tile([128, 3 * QR], f32r, tag=f"w{cc}", name=f"w{cc}")
                    nc.sync.dma_start(
                        w[:], wqkvT[128 * cc : 128 * (cc + 1), :].bitcast(f32r)
                    )
                    wtiles[cc] = w
                ct = tp.tile([128, T], f32, tag="cos")
                st_ = tp.tile([128, T], f32, tag="sin")
                nc.sync.dma_start(ct[:], cosT[:])
                nc.sync.dma_start(st_[:], sinT[:])

                ones8 = tp.tile([128, HPC], f32, tag="ones8")
                nc.gpsimd.memset(ones8[:], 1.0)
                for t in range(NKT):
                    var = v_aug[t].rearrange("p (h d) -> p h d", h=HPC)
                    nc.vector.tensor_copy(var[:, :, 64:65], ones8[:].unsqueeze(2))

                for ts in range(NTS):
                    if ts == 0:
                        xts = xts0
                    else:
                        xts = []
                        for cc in range(CC):
                            xt = xp.tile([128, TS], f32r, tag="xts", name=f"x{ts}{cc}")
                            nc.sync.dma_start(
                                xt[:],
                                xT[
                                    128 * cc : 128 * (cc + 1), TS * ts : TS * (ts + 1)
                                ].bitcast(f32r),
                            )
                            xts.append(xt)

                    # q/k rows -> rope (written straight into persistent tiles)
                    for part in range(2):  # 0=q, 1=k
                        for grp in range(2):  # local heads 4*grp .. 4*grp+3
                            ptiles = []
                            for half in range(2):  # x1, x2
                                p = psA.tile(
                                    [128, TS], f32, tag="proj", name=f"p{ts}{part}{grp}{half}"
                                )
                                col0 = QR * part + 256 * grp + 128 * half
                                for cc in range(CC):
                                    nc.tensor.matmul(
                                        p[:],
                                        wtiles[cc][:, col0 : col0 + 128],
                                        xts[cc][:],
                                        start=(cc == 0),
                                        stop=(cc == CC - 1),
                                    )
                                ptiles.append(p)
                            x1p, x2p = ptiles
                            csl = ct[:, TS * ts : TS * (ts + 1)]
                            ssl = st_[:, TS * ts : TS * (ts + 1)]
                            o1 = qk[(part, grp, 0)][:, TS * ts : TS * (ts + 1)]
                            o2 = qk[(part, grp, 1)][:, TS * ts : TS * (ts + 1)]
                            t1 = rt.tile([128, TS], f32, tag="rt", name=f"t1{ts}{part}{grp}")
                            t2 = rt.tile([128, TS], f32, tag="rt", name=f"t2{ts}{part}{grp}")
                            nc.vector.tensor_mul(t1[:], x1p[:], csl)
                            nc.vector.tensor_mul(t2[:], x2p[:], ssl)
                            nc.vector.tensor_sub(o1, t1[:], t2[:])
                            t3 = rt.tile([128, TS], f32, tag="rt", name=f"t3{ts}{part}{grp}")
                            t4 = rt.tile([128, TS], f32, tag="rt", name=f"t4{ts}{part}{grp}")
                            nc.vector.tensor_mul(t3[:], x1p[:], ssl)
                            nc.vector.tensor_mul(t4[:], x2p[:], csl)
                            nc.vector.tensor_add(o2, t3[:], t4[:])

                    # v projection straight into v_aug
                    for tr4 in range(4):
                        t = 4 * ts + tr4
                        p = psA.tile([128, QR], f32, tag="proj", name=f"pv{ts}{tr4}")
                        for cc in range(CC):
                            nc.tensor.matmul(
                                p[:],
                                xts[cc][:, 128 * tr4 : 128 * (tr4 + 1)],
                                wtiles[cc][:, 2 * QR : 3 * QR],
                                start=(cc == 0),
                                stop=(cc == CC - 1),
                            )
                        var = v_aug[t].rearrange("p (h d) -> p h d", h=HPC)
                        nc.vector.tensor_copy(
                            var[:, :, 0:64],
                            p[:].rearrange("p (h d) -> p h d", h=HPC),
                        )

            # ---------------- phase B: attention ----------------
            with tc.tile_pool(name="pB", bufs=1) as pb:
                yT_all = [
                    pb.tile([128, T], f32r, tag=f"ya{j}", name=f"ya{j}")
                    for j in range(4)
                ]
                mt = pb.tile([128, TS + 128], f32r, tag="mask")
                nc.sync.dma_start(mt[:], maskd[:].bitcast(f32r))
                dn = pb.tile([128, TS], f32, tag="dn")
                rcp = pb.tile([128, TS], f32, tag="rcp")

                with (
                    tc.tile_pool(name="epool", bufs=4) as ep,
                    tc.tile_pool(name="dstage", bufs=2) as dsp,
                    tc.tile_pool(name="rstage", bufs=4) as rsp,
                    tc.tile_pool(name="bstage", bufs=4) as bsp,
                    tc.tile_pool(name="psS", bufs=1, space="PSUM") as psS,
                    tc.tile_pool(name="psY", bufs=1, space="PSUM") as psY,
                ):
                    for g4 in range(2):  # head groups of 4: heads 4*g4..4*g4+3
                        for qi in range(NTS):
                            q0 = TS * qi
                            nkt = 4 * (qi + 1)
                            yTs = [
                                psY.tile(
                                    [65, TS], f32, tag=f"yT{i}", name=f"yT{g4}_{qi}_{i}"
                                )
                                for i in range(4)
                            ]
                            for kt in range(nkt):
                                k0 = 128 * kt
                                sT = psS.tile(
                                    [128, 4 * TS], f32, tag="sT", name=f"sT{g4}_{qi}_{kt}"
                                )
                                for lh4 in range(4):
                                    rb = 32 * lh4
                                    for half in range(2):
                                        nc.tensor.matmul(
                                            sT[:, TS * lh4 : TS * (lh4 + 1)],
                                            qk[(1, g4, half)][rb : rb + 32, k0 : k0 + 128],
                                            qk[(0, g4, half)][rb : rb + 32, q0 : q0 + TS],
                                            start=(half == 0),
                                            stop=(half == 1),
                                            tile_position=(rb, 0),
                                        )
                                eT = ep.tile(
                                    [128, 4 * TS], f32r, tag="eT", name=f"eT{g4}_{qi}_{kt}"
                                )
                                nc.scalar.activation(eT[:], sT[:], EXP, scale=0.125)
                                r = kt - 4 * qi
                                if r >= 0:
                                    # causal: cols < 128*r are fully masked,
                                    # then a 128-wide triangular strip.
                                    w = 128 * (r + 1)
                                    msl = mt[:, TS - 128 * r : TS + 128]
                                    ev = eT[:].rearrange("p (s q) -> p s q", s=4)
                                    nc.gpsimd.tensor_mul(
                                        ev[:, :, 0:w],
                                        ev[:, :, 0:w],
                                        msl.unsqueeze(1).broadcast_to([128, 4, w]),
                                    )
                                for lh4 in range(4):
                                    h = 4 * g4 + lh4
                                    nc.tensor.matmul(
                                        yTs[lh4][:],
                                        v_aug[kt][:, 65 * h : 65 * h + 65],
                                        eT[:, TS * lh4 : TS * (lh4 + 1)],
                                        start=(kt == 0),
                                        stop=(kt == nkt - 1),
                                    )
                            # unnormalized copy + denominator staging
                            for lh4 in range(4):
                                h = 4 * g4 + lh4
                                j, e = h // 2, h % 2
                                ridx = 64 * g4 + 4 * lh4 + qi
                                nc.vector.tensor_copy(
                                    yT_all[j][64 * e : 64 * e + 64, q0 : q0 + TS],
                                    yTs[lh4][0:64, :],
                                )
                                dtmp = dsp.tile(
                                    [65, TS], f32, tag="dt", name=f"dt{h}_{qi}"
                                )
                                nc.vector.tensor_copy(dtmp[64:65, :], yTs[lh4][64:65, :])
                                nc.sync.dma_start(
                                    dn[ridx : ridx + 1, :], dtmp[64:65, :]
                                )

                        # per-group normalization (overlaps next group's attention)
                        r0 = 64 * g4
                        nc.vector.reciprocal(
                            rcp[r0 : r0 + 16, :], dn[r0 : r0 + 16, :]
                        )
                        for lh4 in range(4):
                            h = 4 * g4 + lh4
                            j, e = h // 2, h % 2
                            for qi in range(NTS):
                                ridx = 64 * g4 + 4 * lh4 + qi
                                q0 = TS * qi
                                rtile = rsp.tile(
                                    [1, TS], f32, tag="rr", name=f"rr{h}_{qi}"
                                )
                                nc.sync.dma_start(rtile[:], rcp[ridx : ridx + 1, :])
                                bcS = bsp.tile(
                                    [128, TS], f32, tag="bb", name=f"bb{h}_{qi}"
                                )
                                nc.gpsimd.partition_broadcast(bcS[:], rtile[:])
                                ysl = yT_all[j][64 * e : 64 * e + 64, q0 : q0 + TS]
                                nc.vector.tensor_mul(
                                    ysl, ysl, bcS[64 * e : 64 * e + 64, :]
                                )

                # ---------------- out projection ----------------
                with (
                    tc.tile_pool(name="wopool", bufs=1) as wop,
                    tc.tile_pool(name="ostage", bufs=4) as osp,
                    tc.tile_pool(name="psW", bufs=4, space="PSUM") as psW,
                ):
                    wot = []
                    for cc in range(4):
                        w = wop.tile([128, C], f32r, tag=f"wo{cc}", name=f"wo{cc}")
                        nc.sync.dma_start(
                            w[:], woT[128 * cc : 128 * (cc + 1), :].bitcast(f32r)
                        )
                        wot.append(w)
                    for ts in range(NTS):
                        for co in range(8):
                            p = psW.tile([128, TS], f32, tag="op", name=f"o{ts}{co}")
                            for cc in range(4):
                                nc.tensor.matmul(
                                    p[:],
                                    wot[cc][:, 128 * co : 128 * (co + 1)],
                                    yT_all[cc][:, TS * ts : TS * (ts + 1)],
                                    start=(cc == 0),
                                    stop=(cc == 3),
                                )
                            o = osp.tile([128, TS], f32, tag="os", name=f"os{ts}{co}")
                            nc.scalar.copy(o[:], p[:])
                            nc.sync.dma_start(
                                outT[
                                    128 * co : 128 * (co + 1), TS * ts : TS * (ts + 1)
                                ],
                                o[:],
                            )

    nc.compile()
    return nc


def _get_program():
    if "nc" not in _CACHE:
        _CACHE["nc"] = _build_program()
    return _CACHE["nc"]


def _host_inputs(x, cos, sin, Wqkv, Wo):
    """Build the 8 per-core input maps."""
    # permutation of one head-section's 512 rows (head-relative):
    # row-tile layout [x1 h0-3 | x2 h0-3 | x1 h4-7 | x2 h4-7], 32 rows/block
    perm = []
    for grp in range(2):
        for half in range(2):
            for lh in range(4 * grp, 4 * grp + 4):
                for jj in range(32):
                    perm.append(64 * lh + 2 * jj + half)
    perm = np.asarray(perm)

    cosT4 = np.ascontiguousarray(np.tile(cos.T, (4, 1)).astype(np.float32))
    sinT4 = np.ascontiguousarray(np.tile(sin.T, (4, 1)).astype(np.float32))

    # mask [128, 512+128]: 512 zero cols then a lower-triangular 128 block
    tri = (np.arange(128)[:, None] <= np.arange(128)[None, :]).astype(np.float32)
    maskd = np.ascontiguousarray(
        np.concatenate([np.zeros((128, TS), np.float32), tri], axis=1)
    )

    in_maps = []
    for c in range(NCORES):
        b, g = c // 2, c % 2
        hs0 = HPC * g
        sec = np.arange(QR) + DH * hs0  # this core's rows within a section
        Wq = Wqkv[sec[perm], :]
        Wk = Wqkv[C + sec[perm], :]
        Wv = Wqkv[2 * C + sec, :]
        wqkvT = np.ascontiguousarray(np.concatenate([Wq, Wk, Wv], 0).T)
        woTc = np.ascontiguousarray(Wo[:, sec].T)
        xTb = np.ascontiguousarray(x[b].T)
        in_maps.append(
            {
                "xT": xTb,
                "wqkvT": wqkvT,
                "woT": woTc,
                "cosT": cosT4,
                "sinT": sinT4,
                "maskd": maskd,
            }
        )
    return in_maps


def kernel(x, cos, sin, Wqkv, Wo, _want_profile=False):
    from concourse.bass_utils import run_bass_kernel_spmd

    x = np.asarray(x, dtype=np.float32)
    cos = np.asarray(cos, dtype=np.float32)
    sin = np.asarray(sin, dtype=np.float32)
    Wqkv = np.asarray(Wqkv, dtype=np.float32)
    Wo = np.asarray(Wo, dtype=np.float32)

    nc = _get_program()
    in_maps = _host_inputs(x, cos, sin, Wqkv, Wo)
    res = run_bass_kernel_spmd(nc, in_maps, list(range(NCORES)), trace=_want_profile)
    out = np.empty((B, T, C), dtype=np.float32)
    for b in range(B):
        acc = (
            res.results[2 * b]["outT"].astype(np.float32)
            + res.results[2 * b + 1]["outT"].astype(np.float32)
        )
        out[b] = acc.T
    if _want_profile:
        return out, res
    return out



# revision 20
# speedup vs baseline: 1.9349x; 1.9349x over previous
"""Multi-head attention (RoPE + causal softmax + out-proj) on 8 TRN2 NeuronCores.

Sharding: core c handles batch b = c // 2 and head-half g = c % 2 (8 of 16
heads). Each core computes q/k/v projections for its heads, RoPE, causal
attention, and a partial transposed output projection
outT = (y_heads @ Wo_part.T).T; the host sums the two partials per batch.

Device layout notes (v2):
 - All matmul operands are bf16 (PSUM accumulation stays fp32); host converts
   x / Wqkv / Wo to bf16, halving input DMA and enabling 1-cycle/row matmuls
   at any moving width.
 - q/k weight columns are permuted host-side into pair-interleaved layout:
   each 128-row projection group = 2 heads x [even dims (32) | odd dims (32)].
   RoPE runs as full-width vector ops using a PE permutation matmul (swap of
   32-row blocks) to produce the partner operand, with a sign-folded sin tile.
 - Attention is k-major with K=64 stationary per head (single matmul per
   (head, k-tile)); q-chunks are 256 wide (finer causal granularity).
   Causal masking is a -1e30 bias accumulated into sT via an identity-
   stationary matmul on the diagonal tiles, so exp -> y has no mask step.
 - exp on ScalarE (1/sqrt(dh) folded into the activation scale) writes bf16;
   sT PSUM is double-buffered and emission is software-pipelined as
   s(kt) -> exp(kt) -> y(kt-1) so the PE never idles on the scalar engine.
 - v_aug carries a ones column per head, making row 64 of yT the softmax
   denominator; normalization (reciprocal + partition broadcast + multiply)
   rides the DVE/GpSimd/DMA queues and never blocks the PE.
"""

import numpy as np
import ml_dtypes

BF16 = ml_dtypes.bfloat16

B, T, C, H = 4, 2048, 1024, 16
DH = C // H  # 64
NCORES = 8
HPC = H // 2  # 8 heads per core
QR = HPC * DH  # 512 rows per q/k/v section
TS = 512  # projection t-chunk width
NTS = T // TS  # 4
CC = C // 128  # 8 contraction chunks
QS = 512  # attention q-chunk width
NQS = T // QS  # 4
NKT = T // 128  # 16 k-tiles

_CACHE = {}


def _build_program():
    import concourse.mybir as mybir
    import concourse.tile as tile
    from concourse import bacc

    f32 = mybir.dt.float32
    f32r = mybir.dt.float32r
    bf16 = mybir.dt.bfloat16
    EXP = mybir.ActivationFunctionType.Exp

    nc = bacc.Bacc(trn_type="TRN2")

    xT = nc.dram_tensor("xT", [C, T], bf16, kind="ExternalInput").ap()
    wqkvT = nc.dram_tensor("wqkvT", [C, 3 * QR], bf16, kind="ExternalInput").ap()
    woT = nc.dram_tensor("woT", [QR, C], bf16, kind="ExternalInput").ap()
    cosT = nc.dram_tensor("cosT", [128, T], f32, kind="ExternalInput").ap()
    sinT = nc.dram_tensor("sinT", [128, T], f32, kind="ExternalInput").ap()
    maskb = nc.dram_tensor("maskb", [128, 7 * 128], bf16, kind="ExternalInput").ap()
    eyeb = nc.dram_tensor("eyeb", [128, 128], bf16, kind="ExternalInput").ap()
    p32 = nc.dram_tensor("p32", [128, 128], f32, kind="ExternalInput").ap()
    outT = nc.dram_tensor("outT", [C, T], f32, kind="ExternalOutput").ap()

    with tile.TileContext(nc) as tc:
        with tc.tile_pool(name="persist", bufs=1) as pp:
            # rope'd q/k pair tiles: qk[part][pr] = [128, T] bf16, rows =
            # head 2*pr: [even dims 32 | odd dims 32], head 2*pr+1 likewise.
            qk = [
                [
                    pp.tile([128, T], bf16, tag=f"qk{part}{pr}", name=f"qk{part}{pr}")
                    for pr in range(4)
                ]
                for part in range(2)
            ]
            # v with a ones column per head: [t-chunk 128, 8 * 65] bf16
            v_aug = [
                pp.tile([128, HPC * 65], bf16, tag=f"va{t}", name=f"va{t}")
                for t in range(NKT)
            ]
            # out-proj weights + y accumulator tiles (persistent, written in B)
            wot = [
                pp.tile([128, C], bf16, tag=f"wo{cc}", name=f"wo{cc}")
                for cc in range(4)
            ]
            yT_all = [
                pp.tile([128, T], bf16, tag=f"ya{j}", name=f"ya{j}") for j in range(4)
            ]
            mt = pp.tile([128, 7 * 128], bf16, tag="maskb")
            eye = pp.tile([128, 128], bf16, tag="eyeb")
            psw = pp.tile([128, 128], f32r, tag="p32")
            ct = pp.tile([128, T], f32, tag="cos")
            st_ = pp.tile([128, T], f32, tag="sin")


            # ---------------- phase A: qkv projection + rope ----------------
            with (
                tc.tile_pool(name="wpool", bufs=1) as wp,
                tc.tile_pool(name="xpool", bufs=16) as xp,
                tc.tile_pool(name="pstage", bufs=6) as sp,
                tc.tile_pool(name="ropetmp", bufs=6) as rt,
                tc.tile_pool(name="psA", bufs=4, space="PSUM") as psA,
                tc.tile_pool(name="psB", bufs=2, space="PSUM") as psB,
            ):
                # first weight chunk, then first x chunk set, then the rest —
                # so the first matmul group isn't stuck behind the weights
                wtiles = [None] * CC
                w = wp.tile([128, 3 * QR], bf16, tag="w0", name="w0")
                nc.sync.dma_start(w[:], wqkvT[0:128, :])
                wtiles[0] = w
                xts0 = []
                for cc in range(CC):
                    xt = xp.tile([128, TS], bf16, tag="xts", name=f"x0{cc}")
                    nc.sync.dma_start(xt[:], xT[128 * cc : 128 * (cc + 1), 0:TS])
                    xts0.append(xt)
                for cc in range(1, CC):
                    w = wp.tile([128, 3 * QR], bf16, tag=f"w{cc}", name=f"w{cc}")
                    nc.sync.dma_start(w[:], wqkvT[128 * cc : 128 * (cc + 1), :])
                    wtiles[cc] = w
                nc.sync.dma_start(ct[:], cosT[:])
                nc.sync.dma_start(st_[:], sinT[:])
                nc.sync.dma_start(mt[:], maskb[:])
                nc.sync.dma_start(eye[:], eyeb[:])
                nc.sync.dma_start(psw[:], p32[:].bitcast(f32r))
                for cc in range(4):
                    nc.sync.dma_start(
                        wot[cc][:], woT[128 * cc : 128 * (cc + 1), :]
                    )

                for t in range(NKT):
                    var = v_aug[t].rearrange("p (h d) -> p h d", h=HPC)
                    nc.gpsimd.memset(var[:, :, 64:65], 1.0)

                for ts in range(NTS):
                    if ts == 0:
                        xts = xts0
                    else:
                        xts = []
                        for cc in range(CC):
                            xt = xp.tile([128, TS], bf16, tag="xts", name=f"x{ts}{cc}")
                            nc.sync.dma_start(
                                xt[:],
                                xT[128 * cc : 128 * (cc + 1), TS * ts : TS * (ts + 1)],
                            )
                            xts.append(xt)

                    csl = ct[:, TS * ts : TS * (ts + 1)]
                    ssl = st_[:, TS * ts : TS * (ts + 1)]
                    for part in range(2):  # 0=q, 1=k
                        for pr in range(4):  # pair of heads (2pr, 2pr+1)
                            p = psA.tile(
                                [128, TS], f32, tag="proj", name=f"p{ts}{part}{pr}"
                            )
                            col0 = QR * part + 128 * pr
                            for cc in range(CC):
                                nc.tensor.matmul(
                                    p[:],
                                    wtiles[cc][:, col0 : col0 + 128],
                                    xts[cc][:],
                                    start=(cc == 0),
                                    stop=(cc == CC - 1),
                                )
                            ps = sp.tile(
                                [128, TS], f32r, tag="ps", name=f"ps{ts}{part}{pr}"
                            )
                            nc.scalar.copy(ps[:], p[:])
                            pw = psB.tile(
                                [128, TS], f32, tag="pw", name=f"pw{ts}{part}{pr}"
                            )
                            nc.tensor.matmul(
                                pw[:],
                                psw[:],
                                ps[:],
                                start=True,
                                stop=True,
                            )
                            t1 = rt.tile([128, TS], f32, tag="rt", name=f"t1{ts}{part}{pr}")
                            t2 = rt.tile([128, TS], f32, tag="rt", name=f"t2{ts}{part}{pr}")
                            nc.vector.tensor_mul(t1[:], ps[:], csl)
                            nc.vector.tensor_mul(t2[:], pw[:], ssl)
                            nc.vector.tensor_add(
                                qk[part][pr][:, TS * ts : TS * (ts + 1)], t1[:], t2[:]
                            )

                    # v projection: x chunk stationary -> p[t, v-features]
                    for tr4 in range(4):
                        t = 4 * ts + tr4
                        p = psA.tile([128, QR], f32, tag="proj", name=f"pv{ts}{tr4}")
                        for cc in range(CC):
                            nc.tensor.matmul(
                                p[:],
                                xts[cc][:, 128 * tr4 : 128 * (tr4 + 1)],
                                wtiles[cc][:, 2 * QR : 3 * QR],
                                start=(cc == 0),
                                stop=(cc == CC - 1),
                            )
                        var = v_aug[t].rearrange("p (h d) -> p h d", h=HPC)
                        nc.vector.tensor_copy(
                            var[:, :, 0:64],
                            p[:].rearrange("p (h d) -> p h d", h=HPC),
                        )

            # ---------------- phase B: attention ----------------
            with (
                tc.tile_pool(name="epool", bufs=4) as ep,
                tc.tile_pool(name="dstage", bufs=4) as dsp,
                tc.tile_pool(name="rstage", bufs=4) as rsp,
                tc.tile_pool(name="bstage", bufs=4) as bsp,
                tc.tile_pool(name="psS", bufs=2, space="PSUM") as psS,
                tc.tile_pool(name="psY", bufs=2, space="PSUM") as psY,
            ):
                for pr in range(4):  # head pairs: heads 2pr, 2pr+1
                    for qi in range(NQS):
                        q0 = QS * qi
                        nkt = 4 * (qi + 1)
                        yTs = [
                            psY.tile([65, QS], f32, tag=f"yT{i}", name=f"yT{pr}_{qi}_{i}")
                            for i in range(2)
                        ]
                        pend = None  # (kt, eT) awaiting y-matmuls
                        for kt in range(nkt):
                            k0 = 128 * kt
                            r = kt - 4 * qi
                            sT = psS.tile(
                                [128, 2 * QS], f32, tag="sT", name=f"sT{pr}_{qi}_{kt}"
                            )
                            for lh in range(2):
                                prt0 = 64 * lh
                                diag = r >= 0
                                nc.tensor.matmul(
                                    sT[:, QS * lh : QS * (lh + 1)],
                                    qk[1][pr][prt0 : prt0 + 64, k0 : k0 + 128],
                                    qk[0][pr][prt0 : prt0 + 64, q0 : q0 + QS],
                                    start=True,
                                    stop=not diag,
                                )
                                if diag:
                                    # accumulate -1e30 causal bias over the
                                    # full 512 slice: r full 128-blocks, a
                                    # triangular block, then zero padding
                                    c0 = 128 * (3 - r)
                                    nc.tensor.matmul(
                                        sT[:, QS * lh : QS * (lh + 1)],
                                        eye[:],
                                        mt[:, c0 : c0 + 512],
                                        start=False,
                                        stop=True,
                                    )
                            eT = ep.tile(
                                [128, 2 * QS], bf16, tag="eT", name=f"eT{pr}_{qi}_{kt}"
                            )
                            nc.scalar.activation(eT[:], sT[:], EXP, scale=0.125)
                            if pend is not None:
                                pkt, peT = pend
                                for lh in range(2):
                                    h = 2 * pr + lh
                                    nc.tensor.matmul(
                                        yTs[lh][:],
                                        v_aug[pkt][:, 65 * h : 65 * h + 65],
                                        peT[:, QS * lh : QS * (lh + 1)],
                                        start=(pkt == 0),
                                        stop=False,
                                    )
                            pend = (kt, eT)
                        pkt, peT = pend
                        for lh in range(2):
                            h = 2 * pr + lh
                            nc.tensor.matmul(
                                yTs[lh][:],
                                v_aug[pkt][:, 65 * h : 65 * h + 65],
                                peT[:, QS * lh : QS * (lh + 1)],
                                start=(pkt == 0),
                                stop=True,
                            )
                        # unnormalized copy + denominator staging
                        dn2 = dsp.tile([2, QS], f32, tag="dn", name=f"dn{pr}_{qi}")
                        for lh in range(2):
                            nc.vector.tensor_copy(
                                yT_all[pr][64 * lh : 64 * lh + 64, q0 : q0 + QS],
                                yTs[lh][0:64, :],
                            )
                            dtmp = dsp.tile(
                                [65, QS], f32, tag="dt", name=f"dt{pr}_{lh}_{qi}"
                            )
                            nc.vector.tensor_copy(dtmp[64:65, :], yTs[lh][64:65, :])
                            nc.sync.dma_start(dn2[lh : lh + 1, :], dtmp[64:65, :])
                        # normalization for this (pr, qi) — rides DVE/gpsimd/DMA
                        rcp2 = rsp.tile([2, QS], f32, tag="rcp", name=f"rcp{pr}_{qi}")
                        nc.vector.reciprocal(rcp2[:], dn2[:])
                        for lh in range(2):
                            rtile = rsp.tile(
                                [1, QS], f32, tag="rr", name=f"rr{pr}_{lh}_{qi}"
                            )
                            nc.sync.dma_start(rtile[:], rcp2[lh : lh + 1, :])
                            bcS = bsp.tile(
                                [128, QS], f32, tag="bb", name=f"bb{pr}_{lh}_{qi}"
                            )
                            nc.gpsimd.partition_broadcast(bcS[:], rtile[:])
                            ysl = yT_all[pr][64 * lh : 64 * lh + 64, q0 : q0 + QS]
                            nc.vector.tensor_mul(ysl, ysl, bcS[64 * lh : 64 * lh + 64, :])

            # ---------------- phase C: out projection ----------------
            with (
                tc.tile_pool(name="ostage", bufs=4) as osp,
                tc.tile_pool(name="psW", bufs=4, space="PSUM") as psW,
            ):
                for ts in range(NTS):
                    for co in range(8):
                        p = psW.tile([128, TS], f32, tag="op", name=f"o{ts}{co}")
                        for cc in range(4):
                            nc.tensor.matmul(
                                p[:],
                                wot[cc][:, 128 * co : 128 * (co + 1)],
                                yT_all[cc][:, TS * ts : TS * (ts + 1)],
                                start=(cc == 0),
                                stop=(cc == 3),
                            )
                        o = osp.tile([128, TS], f32, tag="os", name=f"os{ts}{co}")
                        nc.scalar.copy(o[:], p[:])
                        nc.sync.dma_start(
                            outT[128 * co : 128 * (co + 1), TS * ts : TS * (ts + 1)],
                            o[:],
                        )

    nc.compile()
    return nc


def _get_program():
    if "nc" not in _CACHE:
        _CACHE["nc"] = _build_program()
    return _CACHE["nc"]


def _host_inputs(x, cos, sin, Wqkv, Wo):
    """Build the 8 per-core input maps."""
    # q/k head-section permutation (head-relative, 512 rows): pair-interleaved
    # [h0 even dims | h0 odd dims | h1 even | h1 odd | h2 even | ...]
    perm = []
    for lh in range(HPC):
        for par in range(2):
            for jj in range(32):
                perm.append(64 * lh + 2 * jj + par)
    perm = np.asarray(perm)

    cosT4 = np.ascontiguousarray(np.tile(cos.T, (4, 1)).astype(np.float32))
    sT = sin.T.astype(np.float32)
    sinT4 = np.ascontiguousarray(np.concatenate([-sT, sT, -sT, sT], axis=0))

    # causal bias tiles [128, 896] bf16: [F F F Tri Z Z Z], F = all -1e30,
    # Tri[p, c] = -1e30 where c < p else 0; slice [128*(3-r), +512)
    tri = np.where(
        np.arange(128)[None, :] < np.arange(128)[:, None], -1e30, 0.0
    ).astype(np.float32)
    full = np.full((128, 128), -1e30, dtype=np.float32)
    zero = np.zeros((128, 128), dtype=np.float32)
    maskb = np.ascontiguousarray(
        np.concatenate([full, full, full, tri, zero, zero, zero], axis=1)
    ).astype(BF16)

    eyeb = np.eye(128, dtype=np.float32).astype(BF16)
    # 32-block swap permutation: out[m] = in[m ^ 32]
    idx = np.arange(128)
    p32 = np.zeros((128, 128), dtype=np.float32)
    p32[idx ^ 32, idx] = 1.0

    in_maps = []
    for c in range(NCORES):
        b, g = c // 2, c % 2
        hs0 = HPC * g
        sec = np.arange(QR) + DH * hs0  # this core's rows within a section
        Wq = Wqkv[sec[perm], :]
        Wk = Wqkv[C + sec[perm], :]
        Wv = Wqkv[2 * C + sec, :]
        wqkvT = np.ascontiguousarray(np.concatenate([Wq, Wk, Wv], 0).T).astype(BF16)
        woTc = np.ascontiguousarray(Wo[:, sec].T).astype(BF16)
        xTb = np.ascontiguousarray(x[b].T).astype(BF16)
        in_maps.append(
            {
                "xT": xTb,
                "wqkvT": wqkvT,
                "woT": woTc,
                "cosT": cosT4,
                "sinT": sinT4,
                "maskb": maskb,
                "eyeb": eyeb,
                "p32": p32,
            }
        )
    return in_maps


def kernel(x, cos, sin, Wqkv, Wo, _want_profile=False):
    from concourse.bass_utils import run_bass_kernel_spmd

    x = np.asarray(x, dtype=np.float32)
    cos = np.asarray(cos, dtype=np.float32)
    sin = np.asarray(sin, dtype=np.float32)
    Wqkv = np.asarray(Wqkv, dtype=np.float32)
    Wo = np.asarray(Wo, dtype=np.float32)

    nc = _get_program()
    in_maps = _host_inputs(x, cos, sin, Wqkv, Wo)
    res = run_bass_kernel_spmd(nc, in_maps, list(range(NCORES)), trace=_want_profile)
    out = np.empty((B, T, C), dtype=np.float32)
    for b in range(B):
        acc = (
            res.results[2 * b]["outT"].astype(np.float32)
            + res.results[2 * b + 1]["outT"].astype(np.float32)
        )
        out[b] = acc.T
    if _want_profile:
        return out, res
    return out
